# revision 7
# baseline (speedup 1.0000x reference)
"""Masked dense layer  out = tanh(x @ (w*mask_w) + b*mask_b)  on 8 TRN2 cores.

Data-parallel: x is sharded along the batch axis (32768 rows per core);
w/b/mask_w/mask_b are replicated. Per core: partition p owns 256 consecutive
rows; x is streamed in 4 MiB slabs ([128, 16, 512] f32, 32 KiB contiguous per
partition). One fused DVE tensor_tensor_reduce per 512-wide row computes
(x_row * wm) and its free-dim sum in a single 1x pass; the resulting
[128, 256] stage gets one ScalarE Tanh (+bias) and one contiguous DMA out.
"""

import numpy as np

import concourse.bacc as bacc
import concourse.bass as bass
import concourse.tile as tile
from concourse import mybir
from concourse.bass_utils import run_bass_kernel_spmd

N, F = 262144, 512
C = 8                 # cores
R = N // C            # rows per core  = 32768
P = 128               # SBUF partitions
RP = R // P           # rows per partition = 256
T = 16                # rows-per-partition per DMA slab
NCHUNK = RP // T      # 16 slabs per core

_cached_nc = None


def build_bass() -> bass.Bass:
    nc = bacc.Bacc()

    x = nc.declare_dram_parameter("x", [R, F], mybir.dt.float32, isOutput=False)
    w = nc.declare_dram_parameter("w", [F, 1], mybir.dt.float32, isOutput=False)
    b = nc.declare_dram_parameter("b", [1], mybir.dt.float32, isOutput=False)
    mask_w = nc.declare_dram_parameter(
        "mask_w", [F, 1], mybir.dt.int32, isOutput=False
    )
    mask_b = nc.declare_dram_parameter("mask_b", [1], mybir.dt.int32, isOutput=False)
    out = nc.declare_dram_parameter("out", [R, 1], mybir.dt.float32, isOutput=True)

    # partition p <- rows [p*RP, (p+1)*RP); per partition each slab is a
    # contiguous T*F*4 = 32 KiB DRAM run.
    x_r = x[:, :].rearrange("(p r) f -> p r f", p=P)      # [128, 256, 512]
    out_r = out[:, :].rearrange("(p r) one -> p (r one)", p=P)  # [128, 256]

    def bcast(src_handle, count):
        """DRAM AP replicating a contiguous `count`-element vector across P partitions."""
        ap = src_handle[:]
        return bass.AP(tensor=ap.tensor, offset=ap.offset, ap=[[0, P], [1, count]])

    with tile.TileContext(nc) as tc:
        with (
            tc.tile_pool(name="singles", bufs=1) as singles,
            tc.tile_pool(name="slabs", bufs=3) as slabs,
            tc.tile_pool(name="scratch", bufs=2) as scratch,
        ):
            # masked weights, broadcast to all partitions: wm[p, f] = w[f]*mask_w[f]
            wb = singles.tile([P, F], mybir.dt.float32)
            nc.gpsimd.dma_start(out=wb, in_=bcast(w, F))
            mw = singles.tile([P, F], mybir.dt.float32)
            nc.gpsimd.dma_start(out=mw, in_=bcast(mask_w, F))  # i32 -> f32 cast
            wm = singles.tile([P, F], mybir.dt.float32)
            nc.vector.tensor_mul(wm, wb, mw)

            # masked bias, per-partition scalar: bm[p, 0] = b[0]*mask_b[0]
            bb = singles.tile([P, 1], mybir.dt.float32)
            nc.gpsimd.dma_start(out=bb, in_=bcast(b, 1))
            mb = singles.tile([P, 1], mybir.dt.float32)
            nc.gpsimd.dma_start(out=mb, in_=bcast(mask_b, 1))
            bm = singles.tile([P, 1], mybir.dt.float32)
            nc.vector.tensor_mul(bm, bb, mb)

            stage = singles.tile([P, RP], mybir.dt.float32)
            for c in range(NCHUNK):
                slab = slabs.tile([P, T, F], mybir.dt.float32)
                nc.sync.dma_start(out=slab, in_=x_r[:, c * T : (c + 1) * T, :])
                for t in range(T):
                    junk = scratch.tile([P, F], mybir.dt.float32)
                    nc.vector.affine_mul_reduce(
                        out=junk,
                        accum_out=stage[:, c * T + t : c * T + t + 1],
                        in0=slab[:, t, :],
                        in1=wm,
                        scale=1.0,
                        bias=0.0,
                    )

            outt = singles.tile([P, RP], mybir.dt.float32)
            nc.scalar.activation(
                out=outt,
                in_=stage,
                func=mybir.ActivationFunctionType.Tanh,
                bias=bm,
                scale=1.0,
            )
            nc.sync.dma_start(out=out_r, in_=outt)

    nc.finalize()
    return nc


def run_sharded(inputs: dict, **run_kwargs):
    """Shard inputs, run on 8 cores, gather. Returns (output, BassKernelResults)."""
    global _cached_nc
    if _cached_nc is None:
        _cached_nc = build_bass()
    nc = _cached_nc

    x = np.ascontiguousarray(np.asarray(inputs["x"], dtype=np.float32))
    w = np.ascontiguousarray(np.asarray(inputs["w"], dtype=np.float32))
    b = np.ascontiguousarray(np.asarray(inputs["b"], dtype=np.float32))
    mask_w = np.ascontiguousarray(np.asarray(inputs["mask_w"], dtype=np.int32))
    mask_b = np.ascontiguousarray(np.asarray(inputs["mask_b"], dtype=np.int32))

    in_maps = [
        {
            "x": x[i * R : (i + 1) * R],
            "w": w,
            "b": b,
            "mask_w": mask_w,
            "mask_b": mask_b,
        }
        for i in range(C)
    ]
    res = run_bass_kernel_spmd(nc, in_maps, core_ids=list(range(C)), **run_kwargs)
    outs = [res.results[i]["out"] for i in range(C)]
    return np.concatenate(outs, axis=0), res


def kernel(x, w, b, mask_w, mask_b) -> np.ndarray:
    out, _ = run_sharded(
        {"x": x, "w": w, "b": b, "mask_w": mask_w, "mask_b": mask_b}
    )
    return out
